# revision 19
# baseline (speedup 1.0000x reference)
"""Trainium2 Bass kernel for nn_MemoryBankV2 (memory-bank attention block).

Strategy: the memory bank is the *original* (detached) input features, so
batch items are fully independent -> shard batch B=128 across 8 NeuronCores
(16 items / core, stride-8 interleaved), replicate the memory bank; zero
collectives.

Algebraic restructure: since mem is the raw input, fold the k/v
projections out of the per-memory-entry work:
  scores   s = (x Wq^T + bq) (mem Wk^T + bk)^T * scale
             = x (Wq^T Wk * scale) mem^T + (bq Wk * scale) mem^T + rowconst
    -> qc = x @ C + u0 with C = Wq^T Wk * sc precomputed on host; the
       rowconst (q . bk) is softmax-invariant and dropped exactly.
  attn     a (mem Wv^T + bv) = (a mem) Wv^T + bv        (sum_m a = 1)
    -> u = mem^T e accumulated over the bank, then one DxD map by Wv^T,
       divide by Z, add bv.
So the only O(M) matmuls left are scores (memT as lhs) and u (memR as
lhs) -- no k/v projection of the replicated bank at all.

Visibility skipping: core d holds items {8k+d} sorted ascending, so local
row-quarter q (items <= 32q+31) only attends to memory tiles mt < 8(q+1);
scores/e tiles shrink to columns [128*(mt//8), R) and z/u accumulate
per-quarter regions (one accumulation group per psum bank; has_written
bits are per element, start clears bank-wide, so exactly one start/stop
per bank).

Pipelining: LayerNorms are column-quartered (stats -> grouped chain ->
bf16-only epilogue per quarter) with the FFN / next-layer q-projection
emitted per quarter so the PE fills the LN chain latency. The residual
stream is truncated to bf16 at each LN output (all consumers read the
bf16 copy). The final gate folds the item-0 override into the sigmoid
(rank-1 matmul adds +64 to item-0 logits -> g saturates to exactly 1.0),
making the blend 3 all-bf16 DVE ops per feature subtile.
"""

import os
import sys

import numpy as np

sys.path.insert(0, "/opt/trn_rl_repo")

import ml_dtypes  # noqa: E402

import concourse.bass as bass  # noqa: E402
import concourse.mybir as mybir  # noqa: E402
import concourse.tile as tile  # noqa: E402
from concourse import bacc  # noqa: E402
from concourse.bass import ds  # noqa: E402
from concourse.bass_utils import run_bass_kernel_spmd  # noqa: E402

B, T, D, L = 128, 32, 512, 2
NCORES = 8
BLOC = B // NCORES      # 16 items per core
R = BLOC * T            # 512 rows per core
M = B * T               # 4096 memory entries
DT = D // 128           # 4 feature subtiles
FT = (4 * D) // 128     # 16 ffn subtiles
MT = M // 128           # 32 memory subtiles
CW = 128                # LN/FFN column-quarter width
NQ = R // CW            # 4 quarters
GATE_SAT = 64.0         # logit boost that saturates sigmoid to 1.0 (item-0)

F32 = mybir.dt.float32
BF16 = mybir.dt.bfloat16
AF = mybir.ActivationFunctionType
ALU = mybir.AluOpType
BF = ml_dtypes.bfloat16

# params tensor column layout (per layer l, base = 48*l)
#   u0 (=bq@Wk*sc): +0..3, bv: +4..7, g1: +12..15, be1: +16..19,
#   b1: +20..35, b2: +36..39, g2: +40..43, be2: +44..47
# globals: bs: 96..99
P_BQ, P_BV, P_G1, P_BE1, P_B1, P_B2, P_G2, P_BE2 = 0, 4, 12, 16, 20, 36, 40, 44
P_BS = 96
P_COLS = 100


class _LNQ:
    """Column-quartered LayerNorm over the partition axis.

    stats(h): bf16 casts + ones-matmul column sums for quarter h.
    finish(per_h): function-grouped chain (scalar tables loaded once),
    then per quarter: rg/mb + bf16-only epilogue into out_bf, followed by
    the caller's per_h(h) emission (FFN / qc) so PE work pipelines with
    the remaining quarters' chains.
    """

    def __init__(self, nc, ps, sb, x, prm, gcol, bcol, out_bf, onesb, epsln,
                 sq_on_dve=False):
        self.nc, self.ps, self.sb = nc, ps, sb
        self.x, self.prm, self.gcol, self.bcol = x, prm, gcol, bcol
        self.out_bf, self.onesb, self.epsln = out_bf, onesb, epsln
        self.sq_on_dve = sq_on_dve
        self.mu = [None] * NQ
        self.sqs = [None] * NQ

    def stats(self, h):
        """Column sums for quarter h; psums drained to SBUF immediately
        (scalar Identity) so the mm psum pool rotates freely."""
        nc, ps, sb, x = self.nc, self.ps, self.sb, self.x
        cs = ds(h * CW, CW)
        mups = ps.tile([128, CW], F32, tag="mm", bufs=3, name="ln_mu")
        sqps = ps.tile([128, CW], F32, tag="mm", bufs=3, name="ln_sq")
        for a in range(DT):
            xs = sb.tile([128, CW], BF16, tag="sq", bufs=8, name="ln_xst")
            nc.scalar.activation(out=xs, in_=x[:, a, cs], func=AF.Identity)
            nc.tensor.matmul(mups, self.onesb, xs,
                             start=(a == 0), stop=(a == DT - 1))
        for a in range(DT):
            sq = sb.tile([128, CW], BF16, tag="sq", bufs=8, name="ln_sqt")
            if self.sq_on_dve:
                # amid FFN gelus: keep Square off the scalar engine so the
                # gelu activation table stays resident
                nc.vector.tensor_mul(sq, x[:, a, cs], x[:, a, cs])
            else:
                nc.scalar.activation(out=sq, in_=x[:, a, cs], func=AF.Square)
            nc.tensor.matmul(sqps, self.onesb, sq,
                             start=(a == 0), stop=(a == DT - 1))
        self.mu[h] = sb.tile([128, CW], F32, tag="lnmu", bufs=8, name="ln_mub")
        nc.scalar.activation(out=self.mu[h], in_=mups, func=AF.Identity,
                             scale=1.0 / D)
        self.sqs[h] = sb.tile([128, CW], F32, tag="lnmu", bufs=8, name="ln_sqs")
        nc.scalar.activation(out=self.sqs[h], in_=sqps, func=AF.Identity,
                             scale=1.0 / D)

    def finish(self, per_h=None):
        nc, sb, prm = self.nc, self.sb, self.prm
        mu, var, rstd = self.mu, [None] * NQ, [None] * NQ
        mu2 = [None] * NQ
        for h in range(NQ):
            mu2[h] = sb.tile([128, CW], F32, tag="lns", bufs=8, name="ln_mu2")
            nc.scalar.activation(out=mu2[h], in_=mu[h], func=AF.Square)
        for h in range(NQ):
            var[h] = sb.tile([128, CW], F32, tag="lns", bufs=8, name="ln_var")
            nc.vector.tensor_sub(var[h], self.sqs[h], mu2[h])
        sd = [None] * NQ
        for h in range(NQ):
            sd[h] = sb.tile([128, CW], F32, tag="lns", bufs=8, name="ln_sd")
            nc.scalar.activation(out=sd[h], in_=var[h], func=AF.Sqrt,
                                 bias=self.epsln, scale=1.0)
        for h in range(NQ):
            rstd[h] = sb.tile([128, CW], F32, tag="lns", bufs=8, name="ln_rstd")
            nc.vector.reciprocal_approx_fast(out=rstd[h], in_=sd[h])
        for h in range(NQ):
            cs = ds(h * CW, CW)
            rg = sb.tile([128, CW], F32, tag="lnrg", bufs=2, name="ln_rg")
            nc.vector.tensor_scalar(out=rg, in0=rstd[h],
                                    scalar1=prm[:, self.gcol:self.gcol + 1],
                                    scalar2=None, op0=ALU.mult)
            mb = sb.tile([128, CW], F32, tag="lnmb", bufs=2, name="ln_mb")
            nc.vector.tensor_mul(mb, mu[h], rg)
            for a in range(DT):
                t = sb.tile([128, CW], F32, tag="lnt", bufs=4, name="ln_t")
                nc.vector.tensor_mul(t, self.x[:, a, cs], rg)
                nc.vector.scalar_tensor_tensor(
                    out=self.out_bf[:, a, cs], in0=t,
                    scalar=prm[:, self.bcol + a:self.bcol + a + 1],
                    in1=mb, op0=ALU.add, op1=ALU.subtract)
            if per_h is not None:
                per_h(h)


def _build():
    nc = bacc.Bacc("TRN2", target_bir_lowering=False, debug=False)

    memT_d = nc.dram_tensor("memT", [D, M], BF16, kind="ExternalInput").ap()
    memR_d = nc.dram_tensor("memR", [M, D], BF16, kind="ExternalInput").ap()
    xT0bf_d = nc.dram_tensor("xT0bf", [D, R], BF16, kind="ExternalInput").ap()
    bb_d = nc.dram_tensor("b_bcast", [128, R], BF16, kind="ExternalInput").ap()
    iv_d = nc.dram_tensor("item_vals", [128, MT], BF16, kind="ExternalInput").ap()
    fl_d = nc.dram_tensor("flag0", [128, R], BF16, kind="ExternalInput").ap()
    prm_d = nc.dram_tensor("params", [128, P_COLS], F32, kind="ExternalInput").ap()
    cq_d, wv_d, w1_d, w2_d = [], [], [], []
    for l in range(L):
        cq_d.append(nc.dram_tensor(f"cq{l}", [D, D], BF16, kind="ExternalInput").ap())
        wv_d.append(nc.dram_tensor(f"wv{l}", [D, D], BF16, kind="ExternalInput").ap())
        w1_d.append(nc.dram_tensor(f"w1{l}", [D, 4 * D], BF16, kind="ExternalInput").ap())
        w2_d.append(nc.dram_tensor(f"w2{l}", [4 * D, D], BF16, kind="ExternalInput").ap())
    ws_d = nc.dram_tensor("ws", [2 * D, D], BF16, kind="ExternalInput").ap()
    out_d = nc.dram_tensor("outT", [D, R], BF16, kind="ExternalOutput").ap()

    with tile.TileContext(nc) as tc:
        with (
            tc.tile_pool(name="sb", bufs=1) as sb,
            tc.tile_pool(name="ps", bufs=1, space="PSUM") as ps,
        ):
            # --- resident inputs -------------------------------------------------
            # emission order = DMA issue order: qc-l0 inputs first, then the
            # attention-loop inputs, then the 8MB memory bank in m-order
            # chunks so low-mt scores/u can start early
            x0bf = sb.tile([128, DT, R], BF16, tag="x0bf", name="x0bf_sb")
            nc.sync.dma_start(out=x0bf, in_=xT0bf_d.rearrange("(a p) n -> p a n", p=128))
            prm = sb.tile([128, P_COLS], F32, tag="prm", name="prm_sb")
            nc.sync.dma_start(out=prm, in_=prm_d[:, :])
            memT = sb.tile([128, DT, M], BF16, tag="memT", name="memT_sb")
            memR = sb.tile([128, MT, D], BF16, tag="memR", name="memR_sb")

            def load_layer_weights(l, skip_wv=False):
                cqw = sb.tile([128, DT, D], BF16, tag="cq", bufs=2, name="cq_sb")
                wvw = sb.tile([128, DT, D], BF16, tag="wv", bufs=2, name="wv_sb")
                nc.sync.dma_start(out=cqw, in_=cq_d[l].rearrange("(a p) n -> p a n", p=128))
                if not skip_wv:
                    nc.sync.dma_start(out=wvw, in_=wv_d[l].rearrange("(a p) n -> p a n", p=128))
                return cqw, wvw

            layer_w = [load_layer_weights(0, skip_wv=True)]
            bb = sb.tile([128, R], BF16, tag="bb", name="bb_sb")
            nc.sync.dma_start(out=bb, in_=bb_d[:, :])
            iv = sb.tile([128, MT], BF16, tag="iv", name="iv_sb")
            nc.sync.dma_start(out=iv, in_=iv_d[:, :])
            for c in range(4):
                msl = slice(c * 1024, (c + 1) * 1024)
                for a in range(DT):
                    sl = slice(a * 128, (a + 1) * 128)
                    nc.sync.dma_start(out=memT[:, a, msl], in_=memT_d[sl, msl])
                nc.sync.dma_start(
                    out=memR[:, c * 8:(c + 1) * 8, :],
                    in_=memR_d[msl, :].rearrange("(mt p) d -> p mt d", p=128))
                if c == 0:
                    # wv needed only after the attention loop
                    nc.sync.dma_start(
                        out=layer_w[0][1],
                        in_=wv_d[0].rearrange("(a p) n -> p a n", p=128))

            onesb = sb.tile([128, 128], BF16, tag="onesb", name="onesb_sb")
            nc.vector.memset(onesb, 1.0)
            lsat = sb.tile([128, 128], BF16, tag="lsat", name="lsat_sb")
            nc.vector.memset(lsat, GATE_SAT / 128.0)
            epsln = sb.tile([128, 1], F32, tag="epsln", name="epsln_sb")
            nc.vector.memset(epsln, 1e-5)

            # --- layer-0 qc projection (reads x0bf directly) ---------------------
            qcbf = sb.tile([128, DT, R], BF16, tag="qbf", bufs=2, name="qc_sb")
            for j in range(DT):
                qps = ps.tile([128, R], F32, tag="mm", bufs=3, name="q_ps")
                for a in range(DT):
                    nc.tensor.matmul(qps, layer_w[0][0][:, a, ds(j * 128, 128)],
                                     x0bf[:, a, :],
                                     start=(a == 0), stop=(a == DT - 1))
                nc.vector.tensor_scalar(out=qcbf[:, j, :], in0=qps,
                                        scalar1=prm[:, P_BQ + j:P_BQ + j + 1],
                                        scalar2=None, op0=ALU.add)

            xprev_bf = x0bf
            for l in range(L):
                base = 48 * l
                cqw, wvw = layer_w[l]

                # --- attention: sT = mem @ qc^T, u = mem^T e, Z = 1^T e ----------
                # visibility skipping: tile mt only serves columns
                # [128*(mt//8), R); z/u accumulate quarter regions with one
                # accumulation group per psum bank (single start/stop).
                ups = []
                for j in range(DT):
                    upj = ps.tile([128, R], F32, tag=f"attn{j}", bufs=1, name=f"u_ps{j}")
                    ups.append(upj)
                zps = ps.tile([128, R], F32, tag="z", bufs=1, name="z_ps")
                for mt in range(MT):
                    c0 = 128 * (mt // 8)
                    w = R - c0
                    sps = ps.tile([128, R], F32, tag="mm", bufs=3, name="s_ps")
                    for a in range(DT):
                        nc.tensor.matmul(sps[:, 0:w], memT[:, a, ds(mt * 128, 128)],
                                         qcbf[:, a, c0:R],
                                         start=(a == 0), stop=(a == DT - 1))
                    eraw = sb.tile([128, R], BF16, tag="eraw", bufs=2, name="eraw_sb")
                    nc.scalar.activation(out=eraw[:, 0:w], in_=sps[:, 0:w], func=AF.Exp)
                    e = sb.tile([128, R], BF16, tag="e", bufs=4, name="e_sb")
                    nc.vector.scalar_tensor_tensor(out=e[:, 0:w], in0=bb[:, c0:R],
                                                   scalar=iv[:, mt:mt + 1],
                                                   in1=eraw[:, 0:w],
                                                   op0=ALU.is_gt, op1=ALU.mult)
                    first = (mt == 0)
                    last = (mt == MT - 1)
                    for q in range(mt // 8, 4):
                        qc0 = 128 * q
                        off = qc0 - c0
                        nc.tensor.matmul(zps[:, qc0:qc0 + 128], onesb,
                                         e[:, off:off + 128],
                                         start=(first and q == 0), stop=last,
                                         skip_group_check=True)
                        for j in range(DT):
                            nc.tensor.matmul(ups[j][:, qc0:qc0 + 128],
                                             memR[:, mt, ds(j * 128, 128)],
                                             e[:, off:off + 128],
                                             start=(first and q == 0), stop=last,
                                             skip_group_check=True)

                # --- map u by Wv^T, normalize, residual into fresh fp32 x --------
                ubf = sb.tile([128, DT, R], BF16, tag="ubf", bufs=1, name="ubf_sb")
                # casts split across scalar+vector to shorten the PE bubble
                nc.scalar.activation(out=ubf[:, 0, :], in_=ups[0], func=AF.Identity)
                nc.vector.tensor_copy(ubf[:, 1, :], ups[1])
                nc.scalar.activation(out=ubf[:, 2, :], in_=ups[2], func=AF.Identity)
                nc.vector.tensor_copy(ubf[:, 3, :], ups[3])
                zt = sb.tile([128, R], F32, tag="at", bufs=2, name="zt_sb")
                nc.scalar.activation(out=zt, in_=zps, func=AF.Copy, bias=1e-9)
                rz = sb.tile([128, R], F32, tag="rz", bufs=1, name="rz_sb")
                nc.vector.reciprocal_approx_fast(out=rz, in_=zt)
                x = sb.tile([128, DT, R], F32, tag="x", bufs=2, name="x_sb")
                for j in range(DT):
                    aps = ps.tile([128, R], F32, tag="mm", bufs=3, name="a_ps")
                    for a in range(DT):
                        nc.tensor.matmul(aps, wvw[:, a, ds(j * 128, 128)], ubf[:, a, :],
                                         start=(a == 0), stop=(a == DT - 1))
                    at = sb.tile([128, R], F32, tag="at", bufs=2, name="at_sb")
                    nc.vector.tensor_mul(at, aps, rz)
                    nc.vector.scalar_tensor_tensor(out=x[:, j, :], in0=at,
                                                   scalar=prm[:, base + P_BV + j:base + P_BV + j + 1],
                                                   in1=xprev_bf[:, j, :],
                                                   op0=ALU.add, op1=ALU.add)

                # prefetch next layer's weights; gate weights before the tail
                if l + 1 < L:
                    layer_w.append(load_layer_weights(l + 1))
                else:
                    ws0c = sb.tile([128, DT, 512], BF16, tag="wsc", bufs=2,
                                   name="ws0c_sb")
                    nc.sync.dma_start(
                        out=ws0c,
                        in_=ws_d[0:512, :].rearrange("(s p) n -> p s n", p=128))
                    ws1c = sb.tile([128, DT, 512], BF16, tag="wsc", bufs=2,
                                   name="ws1c_sb")
                    nc.sync.dma_start(
                        out=ws1c,
                        in_=ws_d[512:1024, :].rearrange("(s p) n -> p s n", p=128))
                    flbf = sb.tile([128, R], BF16, tag="flbf", name="flbf_sb")
                    nc.sync.dma_start(out=flbf, in_=fl_d[:, :])

                # --- LN1 (quartered) with FFN quarters hooked per quarter --------
                xlnbf = sb.tile([128, DT, R], BF16, tag="xbf", bufs=2, name="xlnbf_sb")
                ln1 = _LNQ(nc, ps, sb, x, prm, base + P_G1, base + P_BE1,
                           xlnbf, onesb, epsln)
                for h in range(NQ):
                    ln1.stats(h)

                # FFN weights stream in 512KB chunks; loads issued during
                # quarter 0's o-loop only (chunks shared across quarters)
                w1c, w2c = {}, {}

                def load_ffn_chunk(og):
                    # all 8 chunk tiles stay resident (re-read by every
                    # column quarter) -- bufs=8 avoids pool deadlock
                    w1c[og] = sb.tile([128, DT, 512], BF16, tag="wc", bufs=8,
                                      name="w1c_sb")
                    nc.sync.dma_start(
                        out=w1c[og],
                        in_=w1_d[l][:, ds(og * 512, 512)].rearrange(
                            "(a p) n -> p a n", p=128))
                    w2c[og] = sb.tile([128, DT, 512], BF16, tag="wc", bufs=8,
                                      name="w2c_sb")
                    nc.sync.dma_start(
                        out=w2c[og],
                        in_=w2_d[l][ds(og * 512, 512), :].rearrange(
                            "(s p) n -> p s n", p=128))

                for og in range(4):
                    load_ffn_chunk(og)

                f2ps = []
                for j in range(DT):
                    fpj = ps.tile([128, R], F32, tag=f"attn{j}", bufs=1, name=f"f2_ps{j}")
                    f2ps.append(fpj)
                x2 = sb.tile([128, DT, R], F32, tag="x", bufs=2, name="x2_sb")
                ln2 = _LNQ(nc, ps, sb, x2, prm, base + P_G2, base + P_BE2,
                           None, onesb, epsln, sq_on_dve=True)  # out_bf set below

                def emit_ffn_quarter(h, l=l, base=base, xlnbf=xlnbf, f2ps=f2ps,
                                     x2=x2, ln2=ln2, w1c=w1c, w2c=w2c):
                    cs = ds(h * CW, CW)

                    def emit_f2(hh, o):
                        for j in range(DT):
                            nc.tensor.matmul(f2ps[j][:, cs],
                                             w2c[o // 4][:, o % 4, ds(j * 128, 128)],
                                             hh,
                                             start=(o == 0), stop=(o == FT - 1),
                                             skip_group_check=True)

                    hq = []
                    for o in range(FT):
                        fps = ps.tile([128, CW], F32, tag="mm", bufs=3, name="f1_ps")
                        for a in range(DT):
                            nc.tensor.matmul(fps,
                                             w1c[o // 4][:, a, ds((o % 4) * 128, 128)],
                                             xlnbf[:, a, cs],
                                             start=(a == 0), stop=(a == DT - 1))
                        hh = sb.tile([128, CW], BF16, tag="h", bufs=8, name="h_sb")
                        nc.scalar.activation(out=hh, in_=fps, func=AF.Gelu,
                                             bias=prm[:, base + P_B1 + o:base + P_B1 + o + 1],
                                             scale=1.0)
                        hq.append((hh, o))
                        if len(hq) > 3:
                            emit_f2(*hq.pop(0))
                    for h_o in hq:
                        emit_f2(*h_o)
                    # residual for this quarter, then LN2 stats immediately
                    for j in range(DT):
                        nc.vector.scalar_tensor_tensor(
                            out=x2[:, j, cs], in0=f2ps[j][:, cs],
                            scalar=prm[:, base + P_B2 + j:base + P_B2 + j + 1],
                            in1=xlnbf[:, j, cs], op0=ALU.add, op1=ALU.add)
                    ln2.stats(h)

                ln1.finish(per_h=emit_ffn_quarter)

                # --- LN2 (quartered); next-layer qc hooked per quarter -----------
                xbf = sb.tile([128, DT, R], BF16, tag="xbf", bufs=2, name="xbf_sb")
                ln2.out_bf = xbf
                if l + 1 < L:
                    qcn = sb.tile([128, DT, R], BF16, tag="qbf", bufs=2, name="qcn_sb")
                    cqn = layer_w[l + 1][0]
                    nbase = 48 * (l + 1)

                    def emit_qc_quarter(h, qcn=qcn, cqn=cqn, nbase=nbase, xbf=xbf):
                        cs = ds(h * CW, CW)
                        for j in range(DT):
                            qps = ps.tile([128, CW], F32, tag="mm", bufs=3, name="qn_ps")
                            for a in range(DT):
                                nc.tensor.matmul(qps, cqn[:, a, ds(j * 128, 128)],
                                                 xbf[:, a, cs],
                                                 start=(a == 0), stop=(a == DT - 1))
                            nc.vector.tensor_scalar(out=qcn[:, j, cs], in0=qps,
                                                    scalar1=prm[:, nbase + P_BQ + j:nbase + P_BQ + j + 1],
                                                    scalar2=None, op0=ALU.add)

                    ln2.finish(per_h=emit_qc_quarter)
                    qcbf = qcn
                    xprev_bf = xbf
                else:
                    ln2.finish(per_h=None)

            # --- gate + item-0 blend -------------------------------------------
            # g = sigmoid(Ws0 x0 + Ws1 x + bs + GATE_SAT*flag0); the rank-1
            # lsat@flbf matmul saturates item-0 rows to g = 1.0 exactly, so
            # out = xbf + g*(x0bf - xbf) needs no separate flag blend.
            for j in range(DT):
                gps = ps.tile([128, R], F32, tag=f"attn{j}", bufs=1,
                              name=f"ga_ps{j}")
                for c in range(DT):
                    nc.tensor.matmul(gps, ws0c[:, c, ds(j * 128, 128)],
                                     x0bf[:, c, :],
                                     start=(c == 0), stop=False,
                                     skip_group_check=True)
                for c in range(DT):
                    nc.tensor.matmul(gps, ws1c[:, c, ds(j * 128, 128)],
                                     xbf[:, c, :],
                                     start=False, stop=False,
                                     skip_group_check=True)
                nc.tensor.matmul(gps, lsat, flbf, start=False, stop=True,
                                 skip_group_check=True)
                g = sb.tile([128, R], BF16, tag="gt", bufs=3, name="g_sb")
                nc.scalar.activation(out=g, in_=gps, func=AF.Sigmoid,
                                     bias=prm[:, P_BS + j:P_BS + j + 1], scale=1.0)
                dx = sb.tile([128, R], BF16, tag="gt", bufs=3, name="dx_sb")
                nc.vector.tensor_sub(dx, x0bf[:, j, :], xbf[:, j, :])
                m2 = sb.tile([128, R], BF16, tag="gt", bufs=3, name="m2_sb")
                nc.vector.tensor_mul(m2, g, dx)
                ov = sb.tile([128, R], BF16, tag="ov", bufs=2, name="ov_sb")
                nc.vector.tensor_add(ov, xbf[:, j, :], m2)
                nc.sync.dma_start(out=out_d[j * 128:(j + 1) * 128, :], in_=ov)

    nc.compile()
    return nc


_NC = None


def _get_nc():
    global _NC
    if _NC is None:
        _NC = _build()
    return _NC


def _core_rows(d):
    """Global row indices handled by core d: items {8k+d}, ascending, so
    local row-quarter q holds items <= 32q+31 (enables visibility skipping
    with a core-uniform program)."""
    return np.concatenate(
        [np.arange(T) + (8 * k + d) * T for k in range(BLOC)])


def _make_in_maps(inputs):
    cog = np.asarray(inputs["cognition_features"], np.float32)
    flat = cog.reshape(M, D)
    cogT = np.ascontiguousarray(flat.T)          # [D, M] fp32
    sc = 1.0 / np.sqrt(np.float32(D))

    common = {"memT": cogT.astype(BF), "memR": flat.astype(BF)}
    for l in range(L):
        Wq = np.asarray(inputs["Wq"][l], np.float32)
        Wk = np.asarray(inputs["Wk"][l], np.float32)
        Wv = np.asarray(inputs["Wv"][l], np.float32)
        common[f"cq{l}"] = np.ascontiguousarray(Wq.T @ Wk * sc).astype(BF)
        common[f"wv{l}"] = np.ascontiguousarray(Wv.T).astype(BF)
        common[f"w1{l}"] = np.ascontiguousarray(np.asarray(inputs["W1"][l], np.float32).T).astype(BF)
        common[f"w2{l}"] = np.ascontiguousarray(np.asarray(inputs["W2"][l], np.float32).T).astype(BF)
    common["ws"] = np.ascontiguousarray(np.asarray(inputs["Ws"], np.float32).T).astype(BF)

    prm = np.zeros((128, P_COLS), np.float32)

    def put(col, vec):
        v = np.asarray(vec, np.float32).reshape(-1, 128)
        for j in range(v.shape[0]):
            prm[:, col + j] = v[j]

    for l in range(L):
        base = 48 * l
        Wk = np.asarray(inputs["Wk"][l], np.float32)
        put(base + P_BQ, np.asarray(inputs["bq"][l], np.float32) @ Wk * sc)
        put(base + P_BV, inputs["bv"][l])
        put(base + P_G1, inputs["ln1_g"][l])
        put(base + P_BE1, inputs["ln1_b"][l])
        put(base + P_B1, inputs["b1"][l])
        put(base + P_B2, inputs["b2"][l])
        put(base + P_G2, inputs["ln2_g"][l])
        put(base + P_BE2, inputs["ln2_b"][l])
    put(P_BS, inputs["bs"])
    common["params"] = prm

    item_of_m = (np.arange(M) // T).astype(np.float32)
    iv = np.ascontiguousarray(item_of_m.reshape(MT, 128).T).astype(BF)
    common["item_vals"] = iv

    in_maps = []
    for d in range(NCORES):
        perm = _core_rows(d)
        b_of_r = (perm // T).astype(np.float32)
        im = dict(common)
        im["xT0bf"] = np.ascontiguousarray(cogT[:, perm]).astype(BF)
        im["b_bcast"] = np.broadcast_to(b_of_r, (128, R)).astype(BF).copy()
        im["flag0"] = np.broadcast_to((b_of_r == 0), (128, R)).astype(BF).copy()
        in_maps.append(im)
    return in_maps


def _run(in_maps, trace=False):
    nc = _get_nc()
    return run_bass_kernel_spmd(nc, in_maps, list(range(NCORES)), trace=trace)


def kernel(**inputs):
    in_maps = _make_in_maps(inputs)
    res = _run(in_maps)
    outT = np.empty((M, D), np.float32)
    for d in range(NCORES):
        outT[_core_rows(d), :] = res.results[d]["outT"].T.astype(np.float32)
    return outT.reshape(B, T, D)


if __name__ == "__main__":
    _build()
    print("build ok")


# revision 21
# speedup vs baseline: 1.3874x; 1.3874x over previous
"""Trainium2 Bass kernel for nn_MemoryBankV2 (memory-bank attention block).

Strategy: the memory bank is the *original* (detached) input features, so
batch items are fully independent -> shard batch B=128 across 8 NeuronCores
(16 items / core, stride-8 interleaved), replicate the memory bank; zero
collectives.

Algebraic restructure: since mem is the raw input, fold the k/v
projections out of the per-memory-entry work:
  scores   s = x (Wq^T Wk * scale) mem^T + (bq Wk * scale) mem^T [+ const]
    -> qc = x @ C + u0 with C precomputed on host; the rowconst (q . bk)
       is softmax-invariant and dropped exactly.
  attn     a (mem Wv^T + bv) = (a mem) Wv^T + bv        (sum_m a = 1)
    -> u = mem^T e accumulated over the bank, then one DxD map by Wv^T.
So the only O(M) matmuls left are scores (memT as lhs) and u (memR as
lhs) -- no k/v projection of the replicated bank at all.

Visibility skipping: core d holds items {8k+d} sorted ascending, so local
row-quarter q (items <= 32q+31) only attends to memory tiles mt < 8(q+1);
scores/e tiles shrink to columns [128*(mt//8), R) and z/u accumulate
per-quarter regions (one accumulation group per psum bank; has_written
bits are per element, start clears bank-wide).

The residual stream is bf16 end-to-end: residual stts write bf16, LN
stats matmuls read it directly (no casts), squares on DVE in 2x bf16
mode, and the LN epilogue (bf16 rg/mb) runs in 2x mode. All matmuls stay
>= 512 free-size (small matmuls pay a ~73ns weight-reload tax). The final
gate folds the item-0 override into the sigmoid (rank-1 matmul adds +64
to item-0 logits -> g saturates to exactly 1.0), making the blend 3
all-bf16 DVE ops per feature subtile.
"""

import os
import sys

import numpy as np

sys.path.insert(0, "/opt/trn_rl_repo")

import ml_dtypes  # noqa: E402

import concourse.bass as bass  # noqa: E402
import concourse.mybir as mybir  # noqa: E402
import concourse.tile as tile  # noqa: E402
from concourse import bacc  # noqa: E402
from concourse.bass import ds  # noqa: E402
from concourse.bass_utils import run_bass_kernel_spmd  # noqa: E402

B, T, D, L = 128, 32, 512, 2
NCORES = 8
BLOC = B // NCORES      # 16 items per core
R = BLOC * T            # 512 rows per core
M = B * T               # 4096 memory entries
DT = D // 128           # 4 feature subtiles
FT = (4 * D) // 128     # 16 ffn subtiles
MT = M // 128           # 32 memory subtiles
GATE_SAT = 64.0         # logit boost that saturates sigmoid to 1.0 (item-0)

F32 = mybir.dt.float32
BF16 = mybir.dt.bfloat16
AF = mybir.ActivationFunctionType
ALU = mybir.AluOpType
BF = ml_dtypes.bfloat16

# params tensor column layout (per layer l, base = 48*l)
#   u0 (=bq@Wk*sc): +0..3, bv: +4..7, g1: +12..15, be1: +16..19,
#   b1: +20..35, b2: +36..39, g2: +40..43, be2: +44..47
# globals: bs: 96..99
P_BQ, P_BV, P_G1, P_BE1, P_B1, P_B2, P_G2, P_BE2 = 0, 4, 12, 16, 20, 36, 40, 44
P_BS = 96
P_COLS = 100


def _layernorm(nc, psum, tmps, x, prm, gcol, bcol, out_bf, onesb, epsln):
    """LN over the partition axis of x ([128, DT, R] bf16, read-only);
    writes the normalized bf16 result to out_bf. Stats matmuls read x
    directly (bf16); squares on DVE (2x mode, keeps scalar tables free);
    psums drained to SBUF immediately; epilogue in 2x bf16 mode."""
    mups = psum.tile([128, R], F32, tag="mm", bufs=3, name="ln_mu")
    for a in range(DT):
        nc.tensor.matmul(mups, onesb, x[:, a, :], start=(a == 0), stop=(a == DT - 1))
    sqps = psum.tile([128, R], F32, tag="mm", bufs=3, name="ln_sq")
    for a in range(DT):
        sq = tmps.tile([128, R], BF16, tag="sq", bufs=2, name="ln_sqt")
        nc.vector.tensor_mul(sq, x[:, a, :], x[:, a, :])
        nc.tensor.matmul(sqps, onesb, sq, start=(a == 0), stop=(a == DT - 1))
    mu = tmps.tile([128, R], F32, tag="lnmu", bufs=2, name="ln_mub")
    nc.scalar.activation(out=mu, in_=mups, func=AF.Identity, scale=1.0 / D)
    sqs = tmps.tile([128, R], F32, tag="lnmu", bufs=2, name="ln_sqs")
    nc.scalar.activation(out=sqs, in_=sqps, func=AF.Identity, scale=1.0 / D)
    mu2 = tmps.tile([128, R], F32, tag="lns", bufs=2, name="ln_mu2")
    nc.scalar.activation(out=mu2, in_=mu, func=AF.Square)
    var = tmps.tile([128, R], F32, tag="lns", bufs=2, name="ln_var")
    nc.vector.tensor_sub(var, sqs, mu2)
    sd = tmps.tile([128, R], F32, tag="lns", bufs=2, name="ln_sd")
    nc.scalar.activation(out=sd, in_=var, func=AF.Sqrt, bias=epsln, scale=1.0)
    rstd = tmps.tile([128, R], F32, tag="lns", bufs=2, name="ln_rstd")
    nc.vector.reciprocal_approx_fast(out=rstd, in_=sd)
    # rg = rstd * gain, mb = mu*rg, both bf16 so the epilogue runs 2x
    rg = tmps.tile([128, R], BF16, tag="lnrg", bufs=1, name="ln_rg")
    nc.vector.tensor_scalar(out=rg, in0=rstd, scalar1=prm[:, gcol:gcol + 1],
                            scalar2=None, op0=ALU.mult)
    mb = tmps.tile([128, R], BF16, tag="lnmb", bufs=1, name="ln_mb")
    nc.vector.tensor_mul(mb, mu, rg)
    # out_bf = (x*rg + b) - mb  (== (x-mu)*rg*g + b), all-bf16 2x ops
    for a in range(DT):
        t = tmps.tile([128, R], BF16, tag="lnt", bufs=2, name="ln_t")
        nc.vector.tensor_mul(t, x[:, a, :], rg)
        nc.vector.scalar_tensor_tensor(out=out_bf[:, a, :], in0=t,
                                       scalar=prm[:, bcol + a:bcol + a + 1],
                                       in1=mb, op0=ALU.add, op1=ALU.subtract)


def _build():
    nc = bacc.Bacc("TRN2", target_bir_lowering=False, debug=False)

    memT_d = nc.dram_tensor("memT", [D, M], BF16, kind="ExternalInput").ap()
    memR_d = nc.dram_tensor("memR", [M, D], BF16, kind="ExternalInput").ap()
    xT0bf_d = nc.dram_tensor("xT0bf", [D, R], BF16, kind="ExternalInput").ap()
    bb_d = nc.dram_tensor("b_bcast", [128, R], BF16, kind="ExternalInput").ap()
    iv_d = nc.dram_tensor("item_vals", [128, MT], BF16, kind="ExternalInput").ap()
    fl_d = nc.dram_tensor("flag0", [128, R], BF16, kind="ExternalInput").ap()
    prm_d = nc.dram_tensor("params", [128, P_COLS], F32, kind="ExternalInput").ap()
    cq_d, wv_d, w1_d, w2_d = [], [], [], []
    for l in range(L):
        cq_d.append(nc.dram_tensor(f"cq{l}", [D, D], BF16, kind="ExternalInput").ap())
        wv_d.append(nc.dram_tensor(f"wv{l}", [D, D], BF16, kind="ExternalInput").ap())
        w1_d.append(nc.dram_tensor(f"w1{l}", [D, 4 * D], BF16, kind="ExternalInput").ap())
        w2_d.append(nc.dram_tensor(f"w2{l}", [4 * D, D], BF16, kind="ExternalInput").ap())
    ws_d = nc.dram_tensor("ws", [2 * D, D], BF16, kind="ExternalInput").ap()
    out_d = nc.dram_tensor("outT", [D, R], BF16, kind="ExternalOutput").ap()

    with tile.TileContext(nc) as tc:
        with (
            tc.tile_pool(name="sb", bufs=1) as sb,
            tc.tile_pool(name="ps", bufs=1, space="PSUM") as ps,
        ):
            # --- resident inputs -------------------------------------------------
            # emission order = DMA issue order: qc-l0 inputs first, then the
            # attention-loop inputs, then the 8MB memory bank in m-order
            # chunks so low-mt scores/u can start early
            x0bf = sb.tile([128, DT, R], BF16, tag="x0bf", name="x0bf_sb")
            nc.sync.dma_start(out=x0bf, in_=xT0bf_d.rearrange("(a p) n -> p a n", p=128))
            prm = sb.tile([128, P_COLS], F32, tag="prm", name="prm_sb")
            nc.sync.dma_start(out=prm, in_=prm_d[:, :])
            memT = sb.tile([128, DT, M], BF16, tag="memT", name="memT_sb")
            memR = sb.tile([128, MT, D], BF16, tag="memR", name="memR_sb")

            def load_layer_weights(l, skip_wv=False):
                cqw = sb.tile([128, DT, D], BF16, tag="cq", bufs=2, name="cq_sb")
                wvw = sb.tile([128, DT, D], BF16, tag="wv", bufs=2, name="wv_sb")
                nc.sync.dma_start(out=cqw, in_=cq_d[l].rearrange("(a p) n -> p a n", p=128))
                if not skip_wv:
                    nc.sync.dma_start(out=wvw, in_=wv_d[l].rearrange("(a p) n -> p a n", p=128))
                return cqw, wvw

            layer_w = [load_layer_weights(0, skip_wv=True)]
            bb = sb.tile([128, R], BF16, tag="bb", name="bb_sb")
            nc.sync.dma_start(out=bb, in_=bb_d[:, :])
            iv = sb.tile([128, MT], BF16, tag="iv", name="iv_sb")
            nc.sync.dma_start(out=iv, in_=iv_d[:, :])
            for c in range(4):
                msl = slice(c * 1024, (c + 1) * 1024)
                for a in range(DT):
                    sl = slice(a * 128, (a + 1) * 128)
                    nc.sync.dma_start(out=memT[:, a, msl], in_=memT_d[sl, msl])
                nc.sync.dma_start(
                    out=memR[:, c * 8:(c + 1) * 8, :],
                    in_=memR_d[msl, :].rearrange("(mt p) d -> p mt d", p=128))
                if c == 0:
                    # wv needed only after the attention loop
                    nc.sync.dma_start(
                        out=layer_w[0][1],
                        in_=wv_d[0].rearrange("(a p) n -> p a n", p=128))

            onesb = sb.tile([128, 128], BF16, tag="onesb", name="onesb_sb")
            nc.vector.memset(onesb, 1.0)
            lsat = sb.tile([128, 128], BF16, tag="lsat", name="lsat_sb")
            nc.vector.memset(lsat, GATE_SAT / 128.0)
            epsln = sb.tile([128, 1], F32, tag="epsln", name="epsln_sb")
            nc.vector.memset(epsln, 1e-5)

            # --- layer-0 qc projection (reads x0bf directly) ---------------------
            qcbf = sb.tile([128, DT, R], BF16, tag="qbf", bufs=2, name="qc_sb")
            for j in range(DT):
                qps = ps.tile([128, R], F32, tag="mm", bufs=3, name="q_ps")
                for a in range(DT):
                    nc.tensor.matmul(qps, layer_w[0][0][:, a, ds(j * 128, 128)],
                                     x0bf[:, a, :],
                                     start=(a == 0), stop=(a == DT - 1))
                nc.vector.tensor_scalar(out=qcbf[:, j, :], in0=qps,
                                        scalar1=prm[:, P_BQ + j:P_BQ + j + 1],
                                        scalar2=None, op0=ALU.add)

            xprev_bf = x0bf
            for l in range(L):
                base = 48 * l
                cqw, wvw = layer_w[l]

                # --- attention: sT = mem @ qc^T, u = mem^T e, Z = 1^T e ----------
                # visibility skipping: tile mt only serves columns
                # [128*(mt//8), R); z/u accumulate quarter regions with one
                # accumulation group per psum bank (single start/stop).
                ups = []
                for j in range(DT):
                    upj = ps.tile([128, R], F32, tag=f"attn{j}", bufs=1, name=f"u_ps{j}")
                    ups.append(upj)
                zps = ps.tile([128, R], F32, tag="z", bufs=1, name="z_ps")
                for mt in range(MT):
                    c0 = 128 * (mt // 8)
                    w = R - c0
                    sps = ps.tile([128, R], F32, tag="mm", bufs=3, name="s_ps")
                    for a in range(DT):
                        nc.tensor.matmul(sps[:, 0:w], memT[:, a, ds(mt * 128, 128)],
                                         qcbf[:, a, c0:R],
                                         start=(a == 0), stop=(a == DT - 1))
                    eraw = sb.tile([128, R], BF16, tag="eraw", bufs=2, name="eraw_sb")
                    nc.scalar.activation(out=eraw[:, 0:w], in_=sps[:, 0:w], func=AF.Exp)
                    e = sb.tile([128, R], BF16, tag="e", bufs=4, name="e_sb")
                    nc.vector.scalar_tensor_tensor(out=e[:, 0:w], in0=bb[:, c0:R],
                                                   scalar=iv[:, mt:mt + 1],
                                                   in1=eraw[:, 0:w],
                                                   op0=ALU.is_gt, op1=ALU.mult)
                    first = (mt == 0)
                    last = (mt == MT - 1)
                    for q in range(mt // 8, 4):
                        qc0 = 128 * q
                        off = qc0 - c0
                        nc.tensor.matmul(zps[:, qc0:qc0 + 128], onesb,
                                         e[:, off:off + 128],
                                         start=(first and q == 0), stop=last,
                                         skip_group_check=True)
                        for j in range(DT):
                            nc.tensor.matmul(ups[j][:, qc0:qc0 + 128],
                                             memR[:, mt, ds(j * 128, 128)],
                                             e[:, off:off + 128],
                                             start=(first and q == 0), stop=last,
                                             skip_group_check=True)

                # --- map u by Wv^T, normalize, residual into bf16 x --------------
                ubf = sb.tile([128, DT, R], BF16, tag="ubf", bufs=1, name="ubf_sb")
                # casts split across scalar+vector to shorten the PE bubble
                nc.scalar.activation(out=ubf[:, 0, :], in_=ups[0], func=AF.Identity)
                nc.vector.tensor_copy(ubf[:, 1, :], ups[1])
                nc.scalar.activation(out=ubf[:, 2, :], in_=ups[2], func=AF.Identity)
                nc.vector.tensor_copy(ubf[:, 3, :], ups[3])
                zt = sb.tile([128, R], F32, tag="at", bufs=2, name="zt_sb")
                nc.scalar.activation(out=zt, in_=zps, func=AF.Copy, bias=1e-9)
                rz = sb.tile([128, R], F32, tag="rz", bufs=1, name="rz_sb")
                nc.vector.reciprocal_approx_fast(out=rz, in_=zt)
                x = sb.tile([128, DT, R], BF16, tag="x", bufs=2, name="x_sb")
                for j in range(DT):
                    aps = ps.tile([128, R], F32, tag="mm", bufs=3, name="a_ps")
                    for a in range(DT):
                        nc.tensor.matmul(aps, wvw[:, a, ds(j * 128, 128)], ubf[:, a, :],
                                         start=(a == 0), stop=(a == DT - 1))
                    at = sb.tile([128, R], F32, tag="at", bufs=2, name="at_sb")
                    nc.vector.tensor_mul(at, aps, rz)
                    nc.vector.scalar_tensor_tensor(out=x[:, j, :], in0=at,
                                                   scalar=prm[:, base + P_BV + j:base + P_BV + j + 1],
                                                   in1=xprev_bf[:, j, :],
                                                   op0=ALU.add, op1=ALU.add)

                # prefetch next layer's weights; gate weights before the tail
                if l + 1 < L:
                    layer_w.append(load_layer_weights(l + 1))
                else:
                    ws0c = sb.tile([128, DT, 512], BF16, tag="wsc", bufs=2,
                                   name="ws0c_sb")
                    nc.sync.dma_start(
                        out=ws0c,
                        in_=ws_d[0:512, :].rearrange("(s p) n -> p s n", p=128))
                    ws1c = sb.tile([128, DT, 512], BF16, tag="wsc", bufs=2,
                                   name="ws1c_sb")
                    nc.sync.dma_start(
                        out=ws1c,
                        in_=ws_d[512:1024, :].rearrange("(s p) n -> p s n", p=128))
                    flbf = sb.tile([128, R], BF16, tag="flbf", name="flbf_sb")
                    nc.sync.dma_start(out=flbf, in_=fl_d[:, :])

                # LN1: bf16 result for the FFN
                xlnbf = sb.tile([128, DT, R], BF16, tag="xbf", bufs=2, name="xlnbf_sb")
                _layernorm(nc, ps, sb, x, prm, base + P_G1, base + P_BE1, xlnbf,
                           onesb, epsln)

                # FFN1 -> FFN2 fused over the 4D dim; FFN2 psums reuse the
                # attn psum banks (freed above). All chunks resident.
                w1c, w2c = {}, {}

                def load_ffn_chunk(og):
                    w1c[og] = sb.tile([128, DT, 512], BF16, tag="wc", bufs=8,
                                      name="w1c_sb")
                    nc.sync.dma_start(
                        out=w1c[og],
                        in_=w1_d[l][:, ds(og * 512, 512)].rearrange(
                            "(a p) n -> p a n", p=128))
                    w2c[og] = sb.tile([128, DT, 512], BF16, tag="wc", bufs=8,
                                      name="w2c_sb")
                    nc.sync.dma_start(
                        out=w2c[og],
                        in_=w2_d[l][ds(og * 512, 512), :].rearrange(
                            "(s p) n -> p s n", p=128))

                for og in range(4):
                    load_ffn_chunk(og)

                f2ps = []
                for j in range(DT):
                    fpj = ps.tile([128, R], F32, tag=f"attn{j}", bufs=1, name=f"f2_ps{j}")
                    f2ps.append(fpj)

                def emit_f2(h, o):
                    for j in range(DT):
                        nc.tensor.matmul(f2ps[j],
                                         w2c[o // 4][:, o % 4, ds(j * 128, 128)], h,
                                         start=(o == 0), stop=(o == FT - 1),
                                         skip_group_check=True)

                # software-pipelined: f2 for step o emitted after FFN1 o+3,
                # hiding the gelu latency from the in-order PE queue
                hq = []
                for o in range(FT):
                    fps = ps.tile([128, R], F32, tag="mm", bufs=3, name="f1_ps")
                    for a in range(DT):
                        nc.tensor.matmul(fps, w1c[o // 4][:, a, ds((o % 4) * 128, 128)],
                                         xlnbf[:, a, :],
                                         start=(a == 0), stop=(a == DT - 1))
                    h = sb.tile([128, R], BF16, tag="h", bufs=4, name="h_sb")
                    nc.scalar.activation(out=h, in_=fps, func=AF.Gelu,
                                         bias=prm[:, base + P_B1 + o:base + P_B1 + o + 1],
                                         scale=1.0)
                    hq.append((h, o))
                    if len(hq) > 3:
                        emit_f2(*hq.pop(0))
                for h_o in hq:
                    emit_f2(*h_o)
                x2 = sb.tile([128, DT, R], BF16, tag="x", bufs=2, name="x2_sb")
                for j in range(DT):
                    nc.vector.scalar_tensor_tensor(out=x2[:, j, :], in0=f2ps[j],
                                                   scalar=prm[:, base + P_B2 + j:base + P_B2 + j + 1],
                                                   in1=xlnbf[:, j, :],
                                                   op0=ALU.add, op1=ALU.add)

                # LN2 + (next layer qc | gate)
                xbf = sb.tile([128, DT, R], BF16, tag="xbf", bufs=2, name="xbf_sb")
                _layernorm(nc, ps, sb, x2, prm, base + P_G2, base + P_BE2, xbf,
                           onesb, epsln)
                if l + 1 < L:
                    qcn = sb.tile([128, DT, R], BF16, tag="qbf", bufs=2, name="qcn_sb")
                    cqn = layer_w[l + 1][0]
                    nbase = 48 * (l + 1)
                    for j in range(DT):
                        qps = ps.tile([128, R], F32, tag="mm", bufs=3, name="qn_ps")
                        for a in range(DT):
                            nc.tensor.matmul(qps, cqn[:, a, ds(j * 128, 128)],
                                             xbf[:, a, :],
                                             start=(a == 0), stop=(a == DT - 1))
                        nc.vector.tensor_scalar(out=qcn[:, j, :], in0=qps,
                                                scalar1=prm[:, nbase + P_BQ + j:nbase + P_BQ + j + 1],
                                                scalar2=None, op0=ALU.add)
                    qcbf = qcn
                    xprev_bf = xbf

            # --- gate + item-0 blend -------------------------------------------
            # g = sigmoid(Ws0 x0 + Ws1 x + bs + GATE_SAT*flag0); the rank-1
            # lsat@flbf matmul saturates item-0 rows to g = 1.0 exactly, so
            # out = xbf + g*(x0bf - xbf) needs no separate flag blend.
            for j in range(DT):
                gps = ps.tile([128, R], F32, tag=f"attn{j}", bufs=1,
                              name=f"ga_ps{j}")
                for c in range(DT):
                    nc.tensor.matmul(gps, ws0c[:, c, ds(j * 128, 128)],
                                     x0bf[:, c, :],
                                     start=(c == 0), stop=False,
                                     skip_group_check=True)
                for c in range(DT):
                    nc.tensor.matmul(gps, ws1c[:, c, ds(j * 128, 128)],
                                     xbf[:, c, :],
                                     start=False, stop=False,
                                     skip_group_check=True)
                nc.tensor.matmul(gps, lsat, flbf, start=False, stop=True,
                                 skip_group_check=True)
                g = sb.tile([128, R], BF16, tag="gt", bufs=3, name="g_sb")
                nc.scalar.activation(out=g, in_=gps, func=AF.Sigmoid,
                                     bias=prm[:, P_BS + j:P_BS + j + 1], scale=1.0)
                dx = sb.tile([128, R], BF16, tag="gt", bufs=3, name="dx_sb")
                nc.vector.tensor_sub(dx, x0bf[:, j, :], xbf[:, j, :])
                m2 = sb.tile([128, R], BF16, tag="gt", bufs=3, name="m2_sb")
                nc.vector.tensor_mul(m2, g, dx)
                ov = sb.tile([128, R], BF16, tag="ov", bufs=2, name="ov_sb")
                nc.vector.tensor_add(ov, xbf[:, j, :], m2)
                nc.sync.dma_start(out=out_d[j * 128:(j + 1) * 128, :], in_=ov)

    nc.compile()
    return nc


_NC = None


def _get_nc():
    global _NC
    if _NC is None:
        _NC = _build()
    return _NC


def _core_rows(d):
    """Global row indices handled by core d: items {8k+d}, ascending, so
    local row-quarter q holds items <= 32q+31 (enables visibility skipping
    with a core-uniform program)."""
    return np.concatenate(
        [np.arange(T) + (8 * k + d) * T for k in range(BLOC)])


def _make_in_maps(inputs):
    cog = np.asarray(inputs["cognition_features"], np.float32)
    flat = cog.reshape(M, D)
    cogT = np.ascontiguousarray(flat.T)          # [D, M] fp32
    sc = 1.0 / np.sqrt(np.float32(D))

    common = {"memT": cogT.astype(BF), "memR": flat.astype(BF)}
    for l in range(L):
        Wq = np.asarray(inputs["Wq"][l], np.float32)
        Wk = np.asarray(inputs["Wk"][l], np.float32)
        Wv = np.asarray(inputs["Wv"][l], np.float32)
        common[f"cq{l}"] = np.ascontiguousarray(Wq.T @ Wk * sc).astype(BF)
        common[f"wv{l}"] = np.ascontiguousarray(Wv.T).astype(BF)
        common[f"w1{l}"] = np.ascontiguousarray(np.asarray(inputs["W1"][l], np.float32).T).astype(BF)
        common[f"w2{l}"] = np.ascontiguousarray(np.asarray(inputs["W2"][l], np.float32).T).astype(BF)
    common["ws"] = np.ascontiguousarray(np.asarray(inputs["Ws"], np.float32).T).astype(BF)

    prm = np.zeros((128, P_COLS), np.float32)

    def put(col, vec):
        v = np.asarray(vec, np.float32).reshape(-1, 128)
        for j in range(v.shape[0]):
            prm[:, col + j] = v[j]

    for l in range(L):
        base = 48 * l
        Wk = np.asarray(inputs["Wk"][l], np.float32)
        put(base + P_BQ, np.asarray(inputs["bq"][l], np.float32) @ Wk * sc)
        put(base + P_BV, inputs["bv"][l])
        put(base + P_G1, inputs["ln1_g"][l])
        put(base + P_BE1, inputs["ln1_b"][l])
        put(base + P_B1, inputs["b1"][l])
        put(base + P_B2, inputs["b2"][l])
        put(base + P_G2, inputs["ln2_g"][l])
        put(base + P_BE2, inputs["ln2_b"][l])
    put(P_BS, inputs["bs"])
    common["params"] = prm

    item_of_m = (np.arange(M) // T).astype(np.float32)
    iv = np.ascontiguousarray(item_of_m.reshape(MT, 128).T).astype(BF)
    common["item_vals"] = iv

    in_maps = []
    for d in range(NCORES):
        perm = _core_rows(d)
        b_of_r = (perm // T).astype(np.float32)
        im = dict(common)
        im["xT0bf"] = np.ascontiguousarray(cogT[:, perm]).astype(BF)
        im["b_bcast"] = np.broadcast_to(b_of_r, (128, R)).astype(BF).copy()
        im["flag0"] = np.broadcast_to((b_of_r == 0), (128, R)).astype(BF).copy()
        in_maps.append(im)
    return in_maps


def _run(in_maps, trace=False):
    nc = _get_nc()
    return run_bass_kernel_spmd(nc, in_maps, list(range(NCORES)), trace=trace)


def kernel(**inputs):
    in_maps = _make_in_maps(inputs)
    res = _run(in_maps)
    outT = np.empty((M, D), np.float32)
    for d in range(NCORES):
        outT[_core_rows(d), :] = res.results[d]["outT"].T.astype(np.float32)
    return outT.reshape(B, T, D)


if __name__ == "__main__":
    _build()
    print("build ok")
